# revision 1
# baseline (speedup 1.0000x reference)
"""CSSA strip-window attention + LePE depthwise conv, Trainium2 Bass kernel.

Config: B=32, H=W=64, C=64, heads=4, head_dim=16, windows 64x8 (512 tokens),
8 windows/image -> 256 windows total, data-parallel: 32 windows per core.

Per-core device kernel, per window (4 heads = one "group"):
  - QK^T as 4 row-tiled matmuls (tile_position packs head_dim=16 contractions
    of 4 heads into the 128x128 PE array), producing transposed scores
    [k, q] in PSUM.
  - exp via ScalarE activation (scale=1/sqrt(hd) folded in), PSUM->SBUF,
    two [128, 1024] tiles per k-chunk (4 heads).
  - AV with stationary [v | ones] (M=17: 16 dims + sumexp row for free),
    4 heads packed as col-tiles into one PSUM bank, accumulated over k-chunks.
  - LePE 3x3 depthwise conv on VectorE via scalar_tensor_tensor taps over a
    zero-padded [66, 10] per-channel image, 2 windows (128 channels) at once.
Host does: window re-layout of q/k/v, and on the way back the softmax
normalization (divide by the sumexp row), LePE add, and inverse re-layout.
"""

import sys

for _p in ("/opt/trn_rl_repo", "/root/.axon_site", "/root/.axon_site/_ro/trn_rl_repo"):
    if _p not in sys.path:
        sys.path.append(_p)

import numpy as np

import concourse.bass as bass
import concourse.mybir as mybir
from concourse.tile import TileContext
from concourse.vector_clock import ScopedClock

B, HW, C, HEADS = 32, 64, 64, 4
HS, WS = 64, 8
HD = C // HEADS            # 16
WIN = HS * WS              # 512
NW = (HW // HS) * (HW // WS)   # 8 windows per image
BW = B * NW                # 256 windows
N_CORES = 8
WPC = BW // N_CORES        # 32 windows per core
SCALE = float(HD) ** -0.5
F32 = mybir.dt.float32
F32R = mybir.dt.float32r
BF16 = mybir.dt.bfloat16
MMDT = BF16  # matmul dtype: BF16 or F32R
AF = mybir.ActivationFunctionType
ALU = mybir.AluOpType


def _install_drain_patch():
    """This container's walrus rejects >1 sync-wait on a Drain ('Too many
    sync wait commands'); split the Tile tail drain's waits across
    single-wait drains."""
    if getattr(TileContext, "_drain_patch_installed", False):
        return

    def _drain_and_barrier(self, tick_clock, wait_clock):
        carrier = self.nc.sync.drain()
        wait_clock.add_sem_waits(
            carrier.ins, ScopedClock({None: tick_clock.global_clock})
        )
        si = carrier.ins.sync_info
        waits = list(si.on_wait) if si is not None else []
        if si is not None and len(waits) > 1:
            si.on_wait = waits[:1]
            for sw in waits[1:]:
                d = self.nc.sync.drain()
                dsi = d.ins.sync_info
                if dsi is None:
                    d.ins.sync_info = mybir.SyncInfo(on_wait=[sw], on_update=[])
                else:
                    dsi.on_wait = [sw]
        self.nc.all_engine_barrier()
        popped = self.nc._tile_sem_poison_stack.pop()
        assert popped is self._sem_poison
        self.nc.clear_and_free_semaphores(list(self.sems.allocated().values()))
        self.nc.all_engine_barrier()

    TileContext._drain_and_barrier = _drain_and_barrier
    TileContext._drain_patch_installed = True


def _r(ap):
    return ap.bitcast(F32R)


def _split_multi_waits(nc):
    """This container's walrus allows only one sync-wait per instruction.
    Hoist extra waits onto same-engine NoOps inserted just before the
    instruction (sequencer processes them in order, so semantics are
    preserved)."""
    for f in nc.m.functions:
        for bb in f.blocks:
            new_insts = []
            for inst in bb.instructions:
                si = inst.sync_info
                waits = list(si.on_wait) if si is not None else []
                if len(waits) > 1:
                    si.on_wait = waits[-1:]
                    for sw in waits[:-1]:
                        nop = mybir.InstNoOp(
                            name=nc.get_next_instruction_name(), ins=[], outs=[]
                        )
                        nop.engine = inst.engine
                        nop.sync_info = mybir.SyncInfo(on_wait=[sw], on_update=[])
                        nc.register_instruction(nop)
                        new_insts.append(nop)
                new_insts.append(inst)
            bb.instructions[:] = new_insts


def build_nc():
    _install_drain_patch()
    nc = bass.Bass("TRN2", target_bir_lowering=False, debug=False,
                   num_devices=N_CORES)

    qk_d = nc.dram_tensor("qkT", [WPC, 128, 2 * WIN], MMDT, kind="ExternalInput")
    va_d = nc.dram_tensor("vaug", [WPC, 128, 4 * HEADS * (HD + 1)], MMDT,
                          kind="ExternalInput")
    vi_d = nc.dram_tensor("vimg", [WPC, C, WIN], F32, kind="ExternalInput")
    wb_d = nc.dram_tensor("wb", [128, 10], F32, kind="ExternalInput")
    oa_d = nc.dram_tensor("out_attn", [WPC, HD + 1, HEADS, WIN], F32,
                          kind="ExternalOutput")
    ol_d = nc.dram_tensor("out_lepe", [WPC, C, WIN], F32, kind="ExternalOutput")

    with TileContext(nc) as tc:
        with (
            tc.tile_pool(name="wpool", bufs=1) as wpool,
            tc.tile_pool(name="qpool", bufs=3) as qpool,
            tc.tile_pool(name="kpool", bufs=3) as kpool,
            tc.tile_pool(name="vapool", bufs=3) as vapool,
            tc.tile_pool(name="ptpool", bufs=6) as ptpool,
            tc.tile_pool(name="vtpool", bufs=2) as vtpool,
            tc.tile_pool(name="laccpool", bufs=2) as laccpool,
            tc.tile_pool(name="stagepool", bufs=8) as stagepool,
            tc.tile_pool(name="qkpool", bufs=1, space=bass.MemorySpace.PSUM) as qkpool,
            tc.tile_pool(name="avpool", bufs=4, space=bass.MemorySpace.PSUM) as avpool,
        ):
            wb_t = wpool.tile([128, 10], F32)
            nc.sync.dma_start(wb_t[:], wb_d.ap())

            def attention_group(w):
                qkt = qpool.tile([128, 2 * WIN], MMDT, tag="qkt")
                nc.sync.dma_start(qkt[:], qk_d.ap()[w])
                # free layout: (c, h, j) — chunk-major, built by host_prep
                va = vapool.tile([128, HEADS * 4 * (HD + 1)], MMDT, tag="va")
                nc.sync.dma_start(va[:], va_d.ap()[w])

                avs = []
                for h in range(HEADS):
                    av_t = avpool.tile([HD + 1, WIN], F32, tag="av")
                    avs.append(av_t)
                pts = [[None, None] for _ in range(4)]
                for c in range(5):
                    if c < 4:
                        # per chunk: two 2-head row-tiled packs into two
                        # 2-bank psum tiles (ping-pong kills the exp WAR gap)
                        for pi in range(2):
                            qk = qkpool.tile([128, 1024], F32, tag="qk")
                            for hh in range(2):
                                h = pi * 2 + hh
                                nc.tensor.matmul(
                                    qk[:, 512 * hh: 512 * (hh + 1)],
                                    lhsT=(qkt[32 * h: 32 * h + HD,
                                               WIN + 128 * c: WIN + 128 * (c + 1)]),
                                    rhs=(qkt[32 * h: 32 * h + HD, 0:WIN]),
                                    start=True, stop=True,
                                    tile_position=(32 * h, 0),
                                )
                            pt = ptpool.tile([128, 1024], BF16, tag="pt")
                            nc.scalar.activation(pt[:], qk[:], AF.Exp, scale=SCALE)
                            pts[c][pi] = pt
                    if c > 0:
                        cp = c - 1
                        for h in range(HEADS):
                            nc.tensor.matmul(
                                avs[h][:, :],
                                lhsT=(va[:, (cp * HEADS + h) * (HD + 1):
                                           (cp * HEADS + h + 1) * (HD + 1)]),
                                rhs=(pts[cp][h // 2][:, 512 * (h % 2):
                                                     512 * (h % 2 + 1)]),
                                start=(cp == 0), stop=(cp == 3),
                            )
                stage = stagepool.tile([HD + 1, HEADS * WIN], F32, tag="stage")
                for h in range(HEADS):
                    nc.vector.tensor_copy(
                        stage[:, WIN * h: WIN * (h + 1)], avs[h][:])
                nc.gpsimd.dma_start(oa_d.ap()[w], stage[:])

            def conv_pair(p):
                vt = vtpool.tile([128, WIN], F32, tag="vt")
                nc.sync.dma_start(
                    vt[:],
                    vi_d.ap().rearrange("w c f -> (w c) f")[
                        (2 * p) * C: (2 * p + 2) * C, :],
                )
                vt3 = vt[:].rearrange("c (h ww) -> c h ww", ww=WS)
                acc = laccpool.tile([128, WIN], F32, tag="lacc")
                acc3 = acc[:].rearrange("c (h ww) -> c h ww", ww=WS)
                # center tap first (full range, fused bias); then clipped taps
                nc.vector.tensor_scalar(
                    out=acc3, in0=vt3,
                    scalar1=wb_t[:, 4:5], scalar2=wb_t[:, 9:10],
                    op0=ALU.mult, op1=ALU.add,
                )
                for tap in range(9):
                    if tap == 4:
                        continue
                    dh, dw = tap // 3 - 1, tap % 3 - 1
                    h0, h1 = max(0, -dh), HS - max(0, dh)
                    w0, w1 = max(0, -dw), WS - max(0, dw)
                    nc.vector.scalar_tensor_tensor(
                        out=acc3[:, h0:h1, w0:w1],
                        in0=vt3[:, h0 + dh: h1 + dh, w0 + dw: w1 + dw],
                        scalar=wb_t[:, tap: tap + 1],
                        in1=acc3[:, h0:h1, w0:w1],
                        op0=ALU.mult, op1=ALU.add,
                    )
                nc.gpsimd.dma_start(
                    ol_d.ap().rearrange("w c f -> (w c) f")[
                        (2 * p) * C: (2 * p + 2) * C, :],
                    acc[:],
                )

            for p in range(WPC // 2):
                conv_pair(p)
                attention_group(2 * p)
                attention_group(2 * p + 1)

    _split_multi_waits(nc)
    return nc


def host_prep(qkv, w_conv, b_conv):
    """Full inputs -> per-core input maps."""
    qkv = np.ascontiguousarray(qkv, dtype=np.float32)
    # [3, B, n, c] -> windows: n = h*64 + ww*8 + ws
    x = qkv.reshape(3, B, HS, HW // WS, WS, C)
    x = x.transpose(0, 1, 3, 2, 4, 5)          # [3, b, ww, h, ws, c]
    x = x.reshape(3, BW, WIN, HEADS, HD)       # [3, win, t, head, d]

    qT = x[0].transpose(0, 2, 3, 1)                         # [win, head, d, t]
    kT = x[1].transpose(0, 2, 3, 1)
    qkT = np.zeros((BW, HEADS, 32, 2 * WIN), dtype=np.float32)
    qkT[:, :, :HD, :WIN] = qT
    qkT[:, :, :HD, WIN:] = kT
    qkT = np.ascontiguousarray(qkT.reshape(BW, 128, 2 * WIN))
    vh = x[2].transpose(0, 2, 1, 3)            # [win, head, t, d]
    va = np.concatenate(
        [vh, np.ones((BW, HEADS, WIN, 1), dtype=np.float32)], axis=3
    )                                          # [win, head, t, 17]
    # device SBUF layout: [win, p=128, (chunk, head, 17)]
    va = va.reshape(BW, HEADS, 4, 128, HD + 1).transpose(0, 3, 2, 1, 4)
    va = np.ascontiguousarray(va.reshape(BW, 128, 4 * HEADS * (HD + 1)))
    # conv image layout: [win, c, t] with t = h*8 + ws
    vi = np.ascontiguousarray(
        x[2].reshape(BW, WIN, C).transpose(0, 2, 1)
    )

    wb = np.zeros((128, 10), dtype=np.float32)
    taps = np.asarray(w_conv, dtype=np.float32).reshape(C, 9)  # [c, dh*3+dw]
    wb[:64, :9] = taps
    wb[64:, :9] = taps
    wb[:64, 9] = np.asarray(b_conv, dtype=np.float32)
    wb[64:, 9] = np.asarray(b_conv, dtype=np.float32)

    if MMDT == BF16:
        import ml_dtypes
        qkT = qkT.astype(ml_dtypes.bfloat16)
        va = va.astype(ml_dtypes.bfloat16)

    in_maps = []
    for core in range(N_CORES):
        s = slice(core * WPC, (core + 1) * WPC)
        in_maps.append({
            "qkT": qkT[s], "vaug": va[s], "vimg": vi[s], "wb": wb,
        })
    return in_maps


def host_post(results):
    """Per-core outputs -> full [B, HW*HW, C] output."""
    oa = np.concatenate([r["out_attn"] for r in results], axis=0)  # [BW,17,4,t]
    oa = oa.transpose(0, 2, 1, 3)                                  # [BW,4,17,t]
    ol = np.concatenate([r["out_lepe"] for r in results], axis=0)  # [BW,64,t]
    num = oa[:, :, :HD, :]
    den = oa[:, :, HD:HD + 1, :]
    y = num / den + ol.reshape(BW, HEADS, HD, WIN)       # [win, head, d, t]
    y = y.reshape(B, NW, C, HS, WS)                      # [b, ww, c, h, ws]
    y = y.transpose(0, 3, 1, 4, 2)                       # [b, h, ww, ws, c]
    return np.ascontiguousarray(y.reshape(B, HW * HW, C))


_NC_CACHE = None


def kernel(qkv, w_conv, b_conv):
    global _NC_CACHE
    from concourse.bass_utils import run_bass_kernel_spmd

    if _NC_CACHE is None:
        _NC_CACHE = build_nc()
    in_maps = host_prep(qkv, w_conv, b_conv)
    res = run_bass_kernel_spmd(
        _NC_CACHE, in_maps, core_ids=list(range(N_CORES)), trace=False
    )
    return host_post(res.results)


if __name__ == "__main__":
    rng = np.random.default_rng(0)
    qkv = rng.standard_normal((3, B, HW * HW, C), dtype=np.float32)
    w_conv = (rng.standard_normal((C, 1, 3, 3)) * 0.1).astype(np.float32)
    b_conv = (rng.standard_normal((C,)) * 0.1).astype(np.float32)
    out = kernel(qkv, w_conv, b_conv)
    print("out", out.shape, out.dtype)



# revision 4
# speedup vs baseline: 1.8013x; 1.8013x over previous
"""CSSA strip-window attention + LePE depthwise conv, Trainium2 Bass kernel.

Config: B=32, H=W=64, C=64, heads=4, head_dim=16, windows 64x8 (512 tokens),
8 windows/image -> 256 windows total, data-parallel: 32 windows per core.

Per-core device kernel, per window (4 heads = one "group"):
  - QK^T as row-tiled matmuls (tile_position packs head_dim=16 contractions
    of 4 heads into the 128x128 PE array), producing transposed scores
    [k, q] in PSUM, two heads per [128, 1024] psum tile.
  - exp via ScalarE activation (scale=1/sqrt(hd) folded in), PSUM->SBUF bf16.
  - AV with stationary [v | ones] (M=17: 16 dims + sumexp row for free),
    4 heads packed as PE column-tiles (tile_position col offsets 0/32/64/96)
    into ONE psum bank at partition offsets 32h, accumulated over k-chunks.
    Output DMAd straight from PSUM (no SBUF staging copy).
  - AV for window w-1 is interleaved into window w's QK chunk stream so the
    PE has work during the WAR waits on the exp drain (pstate ramp).
  - LePE 3x3 depthwise conv on VectorE via scalar_tensor_tensor taps,
    2 windows (128 channels) at once.
Host does: window re-layout of q/k/v, and on the way back the softmax
normalization (divide by the sumexp row), LePE add, and inverse re-layout.
"""

import sys

for _p in ("/opt/trn_rl_repo", "/root/.axon_site", "/root/.axon_site/_ro/trn_rl_repo"):
    if _p not in sys.path:
        sys.path.append(_p)

import numpy as np

import concourse.bass as bass
import concourse.mybir as mybir
from concourse.tile import TileContext
from concourse.vector_clock import ScopedClock

B, HW, C, HEADS = 32, 64, 64, 4
HS, WS = 64, 8
HD = C // HEADS            # 16
WIN = HS * WS              # 512
NW = (HW // HS) * (HW // WS)   # 8 windows per image
BW = B * NW                # 256 windows
N_CORES = 8
WPC = BW // N_CORES        # 32 windows per core
SCALE = float(HD) ** -0.5
F32 = mybir.dt.float32
BF16 = mybir.dt.bfloat16
MMDT = BF16
AF = mybir.ActivationFunctionType
ALU = mybir.AluOpType


def _install_drain_patch():
    """This container's walrus rejects >1 sync-wait on a Drain ('Too many
    sync wait commands'); split the Tile tail drain's waits across
    single-wait drains."""
    if getattr(TileContext, "_drain_patch_installed", False):
        return

    def _drain_and_barrier(self, tick_clock, wait_clock):
        carrier = self.nc.sync.drain()
        wait_clock.add_sem_waits(
            carrier.ins, ScopedClock({None: tick_clock.global_clock})
        )
        si = carrier.ins.sync_info
        waits = list(si.on_wait) if si is not None else []
        if si is not None and len(waits) > 1:
            si.on_wait = waits[:1]
            for sw in waits[1:]:
                d = self.nc.sync.drain()
                dsi = d.ins.sync_info
                if dsi is None:
                    d.ins.sync_info = mybir.SyncInfo(on_wait=[sw], on_update=[])
                else:
                    dsi.on_wait = [sw]
        self.nc.all_engine_barrier()
        popped = self.nc._tile_sem_poison_stack.pop()
        assert popped is self._sem_poison
        self.nc.clear_and_free_semaphores(list(self.sems.allocated().values()))
        self.nc.all_engine_barrier()

    TileContext._drain_and_barrier = _drain_and_barrier
    TileContext._drain_patch_installed = True


def _split_multi_waits(nc):
    """This container's walrus allows only one sync-wait per instruction.
    Hoist extra waits onto same-engine NoOps inserted just before the
    instruction (sequencer processes them in order, so semantics are
    preserved)."""
    for f in nc.m.functions:
        for bb in f.blocks:
            new_insts = []
            for inst in bb.instructions:
                si = inst.sync_info
                waits = list(si.on_wait) if si is not None else []
                if len(waits) > 1:
                    si.on_wait = waits[-1:]
                    for sw in waits[:-1]:
                        nop = mybir.InstNoOp(
                            name=nc.get_next_instruction_name(), ins=[], outs=[]
                        )
                        nop.engine = inst.engine
                        nop.sync_info = mybir.SyncInfo(on_wait=[sw], on_update=[])
                        nc.register_instruction(nop)
                        new_insts.append(nop)
                new_insts.append(inst)
            bb.instructions[:] = new_insts


def build_nc():
    _install_drain_patch()
    nc = bass.Bass("TRN2", target_bir_lowering=False, debug=False,
                   num_devices=N_CORES)

    qk_d = nc.dram_tensor("qkT", [WPC, 128, 2 * WIN], MMDT, kind="ExternalInput")
    va_d = nc.dram_tensor("vaug", [WPC, 128, HEADS * 4 * (HD + 1)], MMDT,
                          kind="ExternalInput")
    vi_d = nc.dram_tensor("vimg", [WPC, C, WIN], F32, kind="ExternalInput")
    wb_d = nc.dram_tensor("wb", [128, 10], F32, kind="ExternalInput")
    # packed AV output: 4 heads at partition offsets 32h, rows 32h..32h+17
    # valid (16 dims + sumexp), rest garbage.
    oa_d = nc.dram_tensor("out_attn", [WPC, 128, WIN], F32,
                          kind="ExternalOutput")
    ol_d = nc.dram_tensor("out_lepe", [WPC, C, WIN], F32, kind="ExternalOutput")

    with TileContext(nc) as tc:
        with (
            tc.tile_pool(name="wpool", bufs=1) as wpool,
            tc.tile_pool(name="qpool", bufs=3) as qpool,
            tc.tile_pool(name="vapool", bufs=3) as vapool,
            tc.tile_pool(name="ptpool", bufs=12) as ptpool,
            tc.tile_pool(name="vtpool", bufs=2) as vtpool,
            tc.tile_pool(name="laccpool", bufs=2) as laccpool,
            tc.tile_pool(name="stagepool", bufs=2) as stagepool,
            tc.tile_pool(name="qkpool", bufs=3, space=bass.MemorySpace.PSUM) as qkpool,
            tc.tile_pool(name="avpool", bufs=2, space=bass.MemorySpace.PSUM) as avpool,
        ):
            wb_t = wpool.tile([128, 10], F32)
            nc.sync.dma_start(wb_t[:], wb_d.ap())

            # per-window state carried between iterations (w-1 pipelining)
            state = {}

            def qk_phase(w, av_cb):
                """QK^T chunks + exp for window w; av_cb() is called between
                chunk tiles to interleave the previous window's AV matmuls
                into the PE stream."""
                qkt = qpool.tile([128, 2 * WIN], MMDT, tag="qkt")
                nc.sync.dma_start(qkt[:], qk_d.ap()[w])
                va = vapool.tile([128, HEADS * 4 * (HD + 1)], MMDT, tag="va")
                nc.sync.dma_start(va[:], va_d.ap()[w])
                pts = [[None, None] for _ in range(4)]
                for c in range(4):
                    for pi in range(2):
                        qk = qkpool.tile([128, 1024], F32, tag="qk")
                        for hh in range(2):
                            h = pi * 2 + hh
                            nc.tensor.matmul(
                                qk[:, 512 * hh: 512 * (hh + 1)],
                                lhsT=(qkt[32 * h: 32 * h + HD,
                                           WIN + 128 * c: WIN + 128 * (c + 1)]),
                                rhs=(qkt[32 * h: 32 * h + HD, 0:WIN]),
                                start=True, stop=True,
                                tile_position=(32 * h, 0),
                            )
                        pt = ptpool.tile([128, 1024], BF16, tag="pt")
                        nc.scalar.activation(pt[:], qk[:], AF.Exp, scale=SCALE)
                        pts[c][pi] = pt
                        av_cb()
                state[w] = (pts, va)

            def av_schedule(w):
                """Returns a callback that issues one (chunk x 4 heads) slice
                of window w's AV matmuls per call; 4 calls total, then DMA."""
                if w < 0:
                    return lambda: None
                pts, va = state.pop(w)
                av = avpool.tile([128, WIN], F32, tag="av")
                step = [0]

                def cb():
                    c = step[0]
                    if c >= 4:
                        return
                    for h in range(HEADS):
                        nc.tensor.matmul(
                            av[32 * h: 32 * h + HD + 1, :],
                            lhsT=(va[:, (c * HEADS + h) * (HD + 1):
                                       (c * HEADS + h + 1) * (HD + 1)]),
                            rhs=(pts[c][h // 2][:, 512 * (h % 2):
                                                 512 * (h % 2 + 1)]),
                            start=(c == 0), stop=(c == 3),
                            tile_position=(0, 32 * h),
                        )
                    step[0] = c + 1
                    if c == 3:
                        stage = stagepool.tile([128, WIN], F32, tag="stage")
                        nc.vector.tensor_copy(stage[:], av[:])
                        nc.gpsimd.dma_start(oa_d.ap()[w], stage[:])
                return cb

            def conv_pair(p):
                vt = vtpool.tile([128, WIN], F32, tag="vt")
                nc.sync.dma_start(
                    vt[:],
                    vi_d.ap().rearrange("w c f -> (w c) f")[
                        (2 * p) * C: (2 * p + 2) * C, :],
                )
                vt3 = vt[:].rearrange("c (h ww) -> c h ww", ww=WS)
                acc = laccpool.tile([128, WIN], F32, tag="lacc")
                acc3 = acc[:].rearrange("c (h ww) -> c h ww", ww=WS)
                # center tap first (full range, fused bias); then clipped taps
                nc.vector.tensor_scalar(
                    out=acc3, in0=vt3,
                    scalar1=wb_t[:, 4:5], scalar2=wb_t[:, 9:10],
                    op0=ALU.mult, op1=ALU.add,
                )
                for tap in range(9):
                    if tap == 4:
                        continue
                    dh, dw = tap // 3 - 1, tap % 3 - 1
                    h0, h1 = max(0, -dh), HS - max(0, dh)
                    w0, w1 = max(0, -dw), WS - max(0, dw)
                    nc.vector.scalar_tensor_tensor(
                        out=acc3[:, h0:h1, w0:w1],
                        in0=vt3[:, h0 + dh: h1 + dh, w0 + dw: w1 + dw],
                        scalar=wb_t[:, tap: tap + 1],
                        in1=acc3[:, h0:h1, w0:w1],
                        op0=ALU.mult, op1=ALU.add,
                    )
                nc.gpsimd.dma_start(
                    ol_d.ap().rearrange("w c f -> (w c) f")[
                        (2 * p) * C: (2 * p + 2) * C, :],
                    acc[:],
                )

            for w in range(WPC):
                if w % 2 == 0:
                    conv_pair(w // 2)
                qk_phase(w, av_schedule(w - 1))
            fin = av_schedule(WPC - 1)
            for _ in range(4):
                fin()

    _split_multi_waits(nc)
    return nc


def host_prep(qkv, w_conv, b_conv):
    """Full inputs -> per-core input maps."""
    qkv = np.ascontiguousarray(qkv, dtype=np.float32)
    # [3, B, n, c] -> windows: n = h*64 + ww*8 + ws
    x = qkv.reshape(3, B, HS, HW // WS, WS, C)
    x = x.transpose(0, 1, 3, 2, 4, 5)          # [3, b, ww, h, ws, c]
    x = x.reshape(3, BW, WIN, HEADS, HD)       # [3, win, t, head, d]

    qT = x[0].transpose(0, 2, 3, 1)                         # [win, head, d, t]
    kT = x[1].transpose(0, 2, 3, 1)
    qkT = np.zeros((BW, HEADS, 32, 2 * WIN), dtype=np.float32)
    qkT[:, :, :HD, :WIN] = qT
    qkT[:, :, :HD, WIN:] = kT
    qkT = np.ascontiguousarray(qkT.reshape(BW, 128, 2 * WIN))
    vh = x[2].transpose(0, 2, 1, 3)            # [win, head, t, d]
    va = np.concatenate(
        [vh, np.ones((BW, HEADS, WIN, 1), dtype=np.float32)], axis=3
    )                                          # [win, head, t, 17]
    # device SBUF layout: [win, p=128, (chunk, head, 17)]
    va = va.reshape(BW, HEADS, 4, 128, HD + 1).transpose(0, 3, 2, 1, 4)
    va = np.ascontiguousarray(va.reshape(BW, 128, 4 * HEADS * (HD + 1)))
    # conv image layout: [win, c, t] with t = h*8 + ws
    vi = np.ascontiguousarray(
        x[2].reshape(BW, WIN, C).transpose(0, 2, 1)
    )

    wb = np.zeros((128, 10), dtype=np.float32)
    taps = np.asarray(w_conv, dtype=np.float32).reshape(C, 9)  # [c, dh*3+dw]
    wb[:64, :9] = taps
    wb[64:, :9] = taps
    wb[:64, 9] = np.asarray(b_conv, dtype=np.float32)
    wb[64:, 9] = np.asarray(b_conv, dtype=np.float32)

    if MMDT == BF16:
        import ml_dtypes
        qkT = qkT.astype(ml_dtypes.bfloat16)
        va = va.astype(ml_dtypes.bfloat16)

    in_maps = []
    for core in range(N_CORES):
        s = slice(core * WPC, (core + 1) * WPC)
        in_maps.append({
            "qkT": qkT[s], "vaug": va[s], "vimg": vi[s], "wb": wb,
        })
    return in_maps


def host_post(results):
    """Per-core outputs -> full [B, HW*HW, C] output."""
    oa = np.concatenate([r["out_attn"] for r in results], axis=0)  # [BW,128,t]
    oa = oa.reshape(BW, HEADS, 32, WIN)                            # [BW,4,32,t]
    ol = np.concatenate([r["out_lepe"] for r in results], axis=0)  # [BW,64,t]
    num = oa[:, :, :HD, :]
    den = oa[:, :, HD:HD + 1, :]
    y = num / den + ol.reshape(BW, HEADS, HD, WIN)       # [win, head, d, t]
    y = y.reshape(B, NW, C, HS, WS)                      # [b, ww, c, h, ws]
    y = y.transpose(0, 3, 1, 4, 2)                       # [b, h, ww, ws, c]
    return np.ascontiguousarray(y.reshape(B, HW * HW, C))


_NC_CACHE = None


def kernel(qkv, w_conv, b_conv):
    global _NC_CACHE
    from concourse.bass_utils import run_bass_kernel_spmd

    if _NC_CACHE is None:
        _NC_CACHE = build_nc()
    in_maps = host_prep(qkv, w_conv, b_conv)
    res = run_bass_kernel_spmd(
        _NC_CACHE, in_maps, core_ids=list(range(N_CORES)), trace=False
    )
    return host_post(res.results)


if __name__ == "__main__":
    rng = np.random.default_rng(0)
    qkv = rng.standard_normal((3, B, HW * HW, C), dtype=np.float32)
    w_conv = (rng.standard_normal((C, 1, 3, 3)) * 0.1).astype(np.float32)
    b_conv = (rng.standard_normal((C,)) * 0.1).astype(np.float32)
    out = kernel(qkv, w_conv, b_conv)
    print("out", out.shape, out.dtype)
